# revision 39
# baseline (speedup 1.0000x reference)
"""Multi-head causal attention on 8 Trainium2 NeuronCores (Bass/Tile).

Problem: B=4, S=1024, D=1024, H=16 heads (dk=64), causal mask, fp32 I/O.

Sharding: 8 cores = 4 batches x 2 head-groups (8 heads each).
  Wq/Wk/Wv sharded column-wise by head (tensor parallel), Wo row-wise;
  the Wo all-reduce is a host-side pairwise sum (2 cores per batch).

v2: software-pipelined single-pass schedule. The PE stream is packed so
it (almost) never waits on the ACT exp pipeline or DMA:
  stage 1   Q,K projections for the first seq half (q-half 0 scores only
            need them), V projection for k-tiles 0-3.
  stage 2   q-half-0 attention steps; PE filler between steps: Q/K
            projections for seq half 1 and V projection k-tiles 4-7.
  stage 3   q-half-1 attention steps; PE filler: normalize + output
            projection + stores for q-half 0.
  stage 4   normalize + output projection + stores for q-half 1.
Engine split: ACT = exp + stage-1 PSUM copies; DVE = all other copies,
mask multiplies, normalize; Pool = constant memsets (hoisted out of the
repeat loop); input loads ride the SP HWDGE queue set, stores + small
staging DMAs ride the ACT queue set so loads never queue behind stores.
Odd-head attnV outputs land on PSUM partitions 63:128 (V packed as
[den, d0..63]) so headout extraction is a partition-aligned copy instead
of an SBUF round-trip DMA.
"""

from contextlib import ExitStack

import ml_dtypes
import numpy as np

import concourse.bacc as bacc
import concourse.tile as tile
from concourse import mybir
from concourse.bass_utils import run_bass_kernel_spmd

F32R = mybir.dt.float32r
F32 = mybir.dt.float32
BF16 = mybir.dt.bfloat16
EXP = mybir.ActivationFunctionType.Exp

S = 1024  # sequence length
D = 1024  # model dim
DK = 64  # head dim
HPC = 8  # heads per core
N_CORES = 8
SCALE = 1.0 / np.sqrt(DK)  # folded into the exp activation


def _emit_setup(nc, tc, t, tl):
    """Constants: loaded/initialized once, outside the repeat loop."""
    nc.sync.dma_start(out=tl["sel2"], in_=t["sel2"][:, :])
    nc.sync.dma_start(out=tl["maskd"], in_=t["maskd"][:, :])
    nc.gpsimd.memset(tl["qtz"].rearrange("p a b -> p (a b)"), 0.0)
    v4 = tl["v_sb"].rearrange("p a b c -> p (a b) c")
    nc.gpsimd.memset(v4[:, :, 64:65], 1.0)  # den col 64 (all heads)


def _emit_iter(nc, tc, t, tl, pools):
    xpool, wpool, epool, xtr, osb, pps, scp, avp = pools
    qtz, kt_sb, v_sb, hout_sb = tl["qtz"], tl["kt_sb"], tl["v_sb"], tl["hout_sb"]
    maskd, sel2, wo_sb = tl["maskd"], tl["sel2"], tl["wo_sb"]

    xq = xpool.tile([128, 8, S], BF16, tag="x")
    xk = xpool.tile([128, 8, S], BF16, tag="x")
    xv = xpool.tile([128, 8, S], BF16, tag="x")
    wq = wpool.tile([128, 8, 512], BF16, tag="w")
    wk = wpool.tile([128, 8, 512], BF16, tag="w")
    wv = wpool.tile([128, 8, 512], BF16, tag="w")

    # ---- input DMAs on the SP queue set, in consumption order ----
    # Q/K stage-1 pieces are chunked so the c-outer matmuls below can
    # start as soon as the first chunk lands.
    xqd = t["xq_t"].rearrange("(n p) s -> p n s", p=128)
    xkd = t["xk_t"].rearrange("(n p) s -> p n s", p=128)
    xvd = t["xv_t"].rearrange("(n p) s -> p n s", p=128)
    for half in range(2):
        nc.sync.dma_start(
            out=wq[:, 4 * half : 4 * half + 4, :],
            in_=t["wq_t"][:, 2048 * half : 2048 * (half + 1)],
        )
        nc.sync.dma_start(
            out=xq[:, 4 * half : 4 * half + 4, 0:512],
            in_=xqd[:, 4 * half : 4 * half + 4, 0:512],
        )
    nc.sync.dma_start(out=wv.rearrange("p a b -> p (a b)"), in_=t["wv_t"][:, :])
    nc.sync.dma_start(out=xv[:, :, 0:512], in_=xvd[:, :, 0:512])
    for half in range(2):
        nc.sync.dma_start(
            out=wk[:, 4 * half : 4 * half + 4, :],
            in_=t["wk_t"][:, 2048 * half : 2048 * (half + 1)],
        )
        nc.sync.dma_start(
            out=xk[:, 4 * half : 4 * half + 4, 0:512],
            in_=xkd[:, 4 * half : 4 * half + 4, 0:512],
        )
    for x_sb, xd in ((xq, xqd), (xk, xkd), (xv, xvd)):
        nc.sync.dma_start(out=x_sb[:, :, 512:1024], in_=xd[:, :, 512:1024])
    nc.sync.dma_start(
        out=wo_sb.rearrange("p a b -> p (a b)"), in_=t["wo_s"][:, :]
    )

    # ---- projection group emitters ----
    eng2 = (nc.scalar.copy, nc.vector.tensor_copy)

    def proj_copies(which, sj, dtile, ps, copy_eng):
        sjs = slice(512 * sj, 512 * (sj + 1))
        if which == "q":
            copy_eng(qtz[0:64, 2 * dtile, sjs], ps[0:64, :])
            copy_eng(qtz[64:128, 2 * dtile + 1, sjs], ps[64:128, :])
        else:
            copy_eng(kt_sb[:, dtile, sjs], ps)

    def proj_stage1(which):
        # c-outer: 4 PSUM half-tiles accumulate in parallel, so the first
        # matmul only needs the first x/w chunk from DRAM.
        w_sb, x_sb = (wq, xq) if which == "q" else (wk, xk)
        ps01 = scp.tile([128, 2, 512], F32, tag="sc")
        ps23 = scp.tile([128, 2, 512], F32, tag="sc")
        for c in range(8):
            for dt in range(4):
                pst = ps01 if dt < 2 else ps23
                nc.tensor.matmul(
                    pst[:, dt % 2, :],
                    w_sb[:, c, 128 * dt : 128 * (dt + 1)],
                    x_sb[:, c, 0:512],
                    start=(c == 0),
                    stop=(c == 7),
                )
        for dt in range(4):
            pst = ps01 if dt < 2 else ps23
            proj_copies(which, 0, dt, pst[:, dt % 2, :], eng2[dt % 2])

    def proj_group(which, sj, dtile, copy_eng):
        w_sb, x_sb = (wq, xq) if which == "q" else (wk, xk)
        ps = pps.tile([128, 512], F32, tag="ps")
        for c in range(8):
            nc.tensor.matmul(
                ps,
                w_sb[:, c, 128 * dtile : 128 * (dtile + 1)],
                x_sb[:, c, 512 * sj : 512 * (sj + 1)],
                start=(c == 0),
                stop=(c == 7),
            )
        proj_copies(which, sj, dtile, ps, copy_eng)

    def vproj_group(stile, copy_eng):
        ps = pps.tile([128, 512], F32, tag="ps")
        for c in range(8):
            nc.tensor.matmul(
                ps,
                xv[:, c, 128 * stile : 128 * (stile + 1)],
                wv[:, c, :],
                start=(c == 0),
                stop=(c == 7),
            )
        copy_eng(
            v_sb[:, stile, :, 0:64],
            ps.rearrange("p (h c) -> p h c", c=64),
        )

    # ---- stage 1: Q seq-half 0, V k-tiles 0-3, K seq-half 0 ----
    # (ordered to match the serialized DMA arrival order above)
    proj_stage1("q")
    for stile in range(4):
        vproj_group(stile, eng2[stile % 2])
    proj_stage1("k")

    # ---- attention steps with PE fillers ----
    def emit_score(qj, hc, ki):
        b = 128 * max(0, ki - 4 * qj)
        kis = slice(128 * ki, 128 * (ki + 1))
        sc = scp.tile([128, 2, 512], F32, tag="sc")
        nc.tensor.matmul(
            sc[:, 0, b:512],
            kt_sb[:, hc, kis],
            qtz[:, 2 * hc, 512 * qj + b : 512 * (qj + 1)],
            start=True,
            stop=True,
        )
        nc.tensor.matmul(
            sc[:, 1, b:512],
            kt_sb[:, hc, kis],
            qtz[:, 2 * hc + 1, 512 * qj + b : 512 * (qj + 1)],
            start=True,
            stop=True,
        )
        return sc

    rects = {}

    def norm_group(qj, hc):
        qsl = slice(512 * qj, 512 * (qj + 1))
        bp = pps.tile([128, 512], F32, tag="ps")
        nc.tensor.matmul(
            bp, sel2[:, :], rects[(qj, hc)], start=True, stop=True
        )
        nc.vector.tensor_mul(hout_sb[:, hc, qsl], hout_sb[:, hc, qsl], bp)

    osbs = {}

    def store(stile, ej, o_sb):
        nc.sync.dma_start(
            out=t["out_p"][
                128 * stile : 128 * (stile + 1), 512 * ej : 512 * (ej + 1)
            ],
            in_=o_sb,
        )

    def oproj_group(stile, ej, copy_eng=None):
        fp = pps.tile([128, 512], F32, tag="ps")
        for hc in range(4):
            nc.tensor.matmul(
                fp,
                hout_sb[:, hc, 128 * stile : 128 * (stile + 1)],
                wo_sb[:, hc, 512 * ej : 512 * (ej + 1)],
                start=(hc == 0),
                stop=(hc == 3),
            )
        o_sb = osb.tile([128, 512], BF16, tag="out")
        (copy_eng or nc.vector.tensor_copy)(o_sb, fp)
        store(stile, ej, o_sb)

    def oproj_a(stile, ej):
        # heads 0-2 partial: runs as late filler during the last q-half-1
        # attention steps (head-pair 3 is still being accumulated then)
        fp = pps.tile([128, 512], F32, tag="ps")
        for hc in range(3):
            nc.tensor.matmul(
                fp,
                hout_sb[:, hc, 128 * stile : 128 * (stile + 1)],
                wo_sb[:, hc, 512 * ej : 512 * (ej + 1)],
                start=(hc == 0),
                stop=(hc == 2),
            )
        o_sb = osb.tile([128, 512], BF16, tag="out")
        eng2[ej](o_sb, fp)
        osbs[(stile, ej)] = o_sb

    def oproj_b(stile, ej):
        # head-pair 3 contribution + combine + store (the only tail work)
        fp = pps.tile([128, 512], F32, tag="ps")
        nc.tensor.matmul(
            fp,
            hout_sb[:, 3, 128 * stile : 128 * (stile + 1)],
            wo_sb[:, 3, 512 * ej : 512 * (ej + 1)],
            start=True,
            stop=True,
        )
        o_sb = osbs.pop((stile, ej))
        nc.vector.tensor_add(o_sb, o_sb, fp)
        store(stile, ej, o_sb)

    steps = []
    for qj in range(2):
        kmax = 4 if qj == 0 else 8
        for hc in range(4):
            for ki in range(kmax):
                steps.append((qj, hc, ki, kmax))

    # filler schedule: {step index: [closure, ...]}
    fillers = {}

    def add_filler(i, f):
        fillers.setdefault(i, []).append(f)

    # filler order tracks DMA arrival order: xq half 1, xk half 1,
    # xv half 1 (transfers serialize on the DMA engines)
    for j in range(4):  # steps 0-3: Q seq-half 1
        add_filler(j, lambda d=j: proj_group("q", 1, d, nc.vector.tensor_copy))
    for j in range(4):  # steps 4-7: K seq-half 1
        add_filler(
            4 + j, lambda d=j: proj_group("k", 1, d, nc.vector.tensor_copy)
        )
    for j in range(4):  # steps 8-11: V k-tiles 4-7
        add_filler(8 + j, lambda s=4 + j: vproj_group(s, nc.vector.tensor_copy))
    for j in range(4):  # steps 17,19,21,23: normalize q-half 0
        add_filler(17 + 2 * j, lambda h=j: norm_group(0, h))
    for j in range(8):  # steps 25,27,..,39: output projection q-half 0
        add_filler(
            25 + 2 * j,
            lambda s=j // 2, e=j % 2: oproj_group(s, e, nc.scalar.copy),
        )
    # normalize q-half 1 head-pairs 0-2 right after their extraction
    add_filler(26, lambda: norm_group(1, 0))
    add_filler(34, lambda: norm_group(1, 1))
    add_filler(42, lambda: norm_group(1, 2))
    for j in range(8):  # steps 43-46: heads 0-2 partial outproj, q-half 1
        add_filler(43 + j // 2, lambda s=4 + j // 2, e=j % 2: oproj_a(s, e))

    sc_next = emit_score(*steps[0][:3])
    avs = {}
    for i, (qj, hc, ki, kmax) in enumerate(steps):
        qsl = slice(512 * qj, 512 * (qj + 1))
        if ki == 0:
            av_e = avp.tile([128, 512], F32, tag="av")
            av_o = avp.tile([128, 512], F32, tag="av")
            avs[(qj, hc)] = (av_e, av_o)
        av_e, av_o = avs[(qj, hc)]
        sc = sc_next
        if i + 1 < len(steps):  # cross-step score lookahead
            sc_next = emit_score(*steps[i + 1][:3])
        for f in fillers.get(i, ()):
            f()
        b = 128 * max(0, ki - 4 * qj)
        ee = epool.tile([128, 2, 512], BF16, tag="e")
        nc.scalar.activation(
            ee[:, :, b:512], sc[:, :, b:512], EXP, scale=float(SCALE)
        )
        if ki - 4 * qj >= 0:  # diagonal block: 0/1 mask
            nc.vector.tensor_mul(
                ee[:, 0, b : b + 128], ee[:, 0, b : b + 128], maskd
            )
            nc.vector.tensor_mul(
                ee[:, 1, b : b + 128], ee[:, 1, b : b + 128], maskd
            )
        nc.tensor.matmul(
            av_e[0:65, b:512],
            v_sb[:, ki, 2 * hc, :],
            ee[:, 0, b:512],
            start=(ki == 0),
            stop=(ki == kmax - 1),
        )
        nc.tensor.matmul(
            av_o[0:65, b:512],
            v_sb[:, ki, 2 * hc + 1, :],
            ee[:, 1, b:512],
            start=(ki == 0),
            stop=(ki == kmax - 1),
        )
        if ki != kmax - 1:
            continue
        del avs[(qj, hc)]
        # denominator reciprocals first (they gate the normalize fillers),
        # then headout^T extraction; the odd half changes partitions
        # (0:64 -> 64:128) via an SBUF round-trip DMA.  The even-half copy
        # rides ACT so the DVE-side chain (otmp -> DMA) starts sooner.
        rec2 = xtr.tile([2, 512], F32R, tag="rect")
        ro_t = xtr.tile([1, 512], F32R, tag="reco")
        with nc.allow_low_precision(reason="softmax reciprocal"):
            nc.vector.reciprocal(rec2[0:1, :], av_e[64:65, :])
            nc.vector.reciprocal(ro_t, av_o[64:65, :])
        # partition 0 -> 1 move must go through DMA (engine partition
        # bases are quadrant-restricted)
        nc.sync.dma_start(out=rec2[1:2, :], in_=ro_t)
        rects[(qj, hc)] = rec2
        otmp = xtr.tile([64, 512], BF16, tag="otmp")
        last = qj == 1 and hc == 3
        # last extraction is tail-critical: otmp -> DMA on the idle ACT,
        # recips in parallel on DVE
        (nc.scalar.copy if last else nc.vector.tensor_copy)(otmp, av_o[0:64, :])
        nc.sync.dma_start(out=hout_sb[64:128, hc, qsl], in_=otmp)
        (nc.vector.tensor_copy if last else nc.scalar.copy)(
            hout_sb[0:64, hc, qsl], av_e[0:64, :]
        )

    # ---- tail: last head-pair normalize + outproj combine, q-half 1 ----
    norm_group(1, 3)
    for j in range(8):
        oproj_b(4 + j // 2, j % 2)


def _build_phases(phases, repeat=1):
    return _build(repeat)


def _build(repeat=1, phases=None):
    nc = bacc.Bacc()
    t = {}
    for name in ("xq_t", "xk_t", "xv_t"):
        t[name] = nc.dram_tensor(name, [D, S], BF16, kind="ExternalInput")
    for name in ("wq_t", "wk_t", "wv_t"):
        t[name] = nc.dram_tensor(name, [128, 8 * 512], BF16, kind="ExternalInput")
    t["wo_s"] = nc.dram_tensor("wo_s", [128, 4 * D], BF16, kind="ExternalInput")
    t["maskd"] = nc.dram_tensor("maskd", [128, 128], BF16, kind="ExternalInput")
    t["sel2"] = nc.dram_tensor("sel2", [2, 128], F32R, kind="ExternalInput")
    t["out_p"] = nc.dram_tensor("out_p", [S, D], BF16, kind="ExternalOutput")

    with tile.TileContext(nc) as tc:
        ctx = ExitStack()
        with ctx:
            main = ctx.enter_context(tc.tile_pool(name="main", bufs=1))
            xpool = ctx.enter_context(tc.tile_pool(name="xin", bufs=3))
            wpool = ctx.enter_context(tc.tile_pool(name="win", bufs=3))
            epool = ctx.enter_context(tc.tile_pool(name="epool", bufs=8))
            xtr = ctx.enter_context(tc.tile_pool(name="xtr", bufs=10))
            osb = ctx.enter_context(tc.tile_pool(name="osb", bufs=9))
            pps = ctx.enter_context(
                tc.tile_pool(name="pps", bufs=2, space="PSUM")
            )
            scp = ctx.enter_context(
                tc.tile_pool(name="scp", bufs=2, space="PSUM")
            )
            avp = ctx.enter_context(
                tc.tile_pool(name="avp", bufs=2, space="PSUM")
            )
            pools = (xpool, wpool, epool, xtr, osb, pps, scp, avp)

            tl = {
                "qtz": main.tile([128, 8, S], BF16, name="qtz"),
                "kt_sb": main.tile([128, 4, S], BF16, name="kt_sb"),
                "v_sb": main.tile([128, 8, 8, 65], BF16, name="v_sb"),
                "hout_sb": main.tile([128, 4, S], BF16, name="hout_sb"),
                "maskd": main.tile([128, 128], BF16, name="maskd"),
                "sel2": main.tile([2, 128], F32R, name="sel2"),
                "wo_sb": main.tile([128, 4, S], BF16, name="wo_sb"),
            }
            _emit_setup(nc, tc, t, tl)
            if repeat == 1:
                _emit_iter(nc, tc, t, tl, pools)
            else:
                with tc.For_i(0, repeat, 1):
                    _emit_iter(nc, tc, t, tl, pools)
    nc.compile()
    return nc


_CACHE = {}


def _get(repeat=1):
    if repeat not in _CACHE:
        _CACHE[repeat] = _build(repeat)
    return _CACHE[repeat]


def _host_prep(query, key, value, mask, Wq, Wk, Wv, Wo):
    """Build the per-core in_maps. Returns None if mask isn't causal tril."""
    m = np.asarray(mask)[0, 0]
    if not np.array_equal(m, np.tril(np.ones((S, S), m.dtype))):
        return None

    bf = ml_dtypes.bfloat16

    # diagonal-block mask (same for every diagonal tile under causal tril)
    maskd = m[0:128, 0:128].T.astype(bf)

    sel2 = np.zeros((2, 128), np.float32)
    sel2[0, 0:64] = 1.0
    sel2[1, 64:128] = 1.0

    def ileave(a):  # [R, C] -> [128, (R//128)*C]: chunk-c data contiguous per p
        R, C = a.shape
        return np.ascontiguousarray(
            a.reshape(R // 128, 128, C).transpose(1, 0, 2).reshape(128, -1)
        )

    in_maps = []
    for c in range(N_CORES):
        b, g = c // 2, c % 2
        gsl = slice(512 * g, 512 * (g + 1))
        in_maps.append(
            {
                "xq_t": np.ascontiguousarray(query[b].T.astype(bf)),
                "xk_t": np.ascontiguousarray(key[b].T.astype(bf)),
                "xv_t": np.ascontiguousarray(value[b].T.astype(bf)),
                "wq_t": ileave(Wq[gsl, :].T.astype(bf)),
                "wk_t": ileave(Wk[gsl, :].T.astype(bf)),
                "wv_t": ileave(Wv[gsl, :].T.astype(bf)),
                "wo_s": ileave(Wo[:, gsl].T.astype(bf)),
                "maskd": maskd,
                "sel2": sel2,
            }
        )
    return in_maps


def _gather(results, bo, B):
    out = np.empty((B, S, D), np.float32)
    for b in range(B):
        out[b] = (
            results[2 * b]["out_p"].astype(np.float32)
            + results[2 * b + 1]["out_p"].astype(np.float32)
            + np.asarray(bo)[None, :]
        )
    return out


def _reference_fallback(query, key, value, mask, Wq, Wk, Wv, Wo, bo):
    B = query.shape[0]
    H = 16
    dk = D // H
    q = np.asarray(query, np.float32)
    k = np.asarray(key, np.float32)
    v = np.asarray(value, np.float32)

    def proj(x, W):
        return (x @ W.T).reshape(B, S, H, dk).transpose(0, 2, 1, 3)

    Q, K, V = proj(q, Wq), proj(k, Wk), proj(v, Wv)
    sc = np.einsum("bhqd,bhkd->bhqk", Q, K) / np.sqrt(np.float32(dk))
    sc = np.where(np.asarray(mask) == 0, np.float32(-1e9), sc)
    sc = sc - sc.max(axis=-1, keepdims=True)
    a = np.exp(sc)
    a = a / a.sum(axis=-1, keepdims=True)
    o = np.einsum("bhqk,bhkd->bhqd", a, V).transpose(0, 2, 1, 3).reshape(B, S, D)
    return (o @ np.asarray(Wo).T + np.asarray(bo)).astype(np.float32)


def kernel(query, key, value, mask, Wq, Wk, Wv, Wo, bo):
    query = np.asarray(query, np.float32)
    key = np.asarray(key, np.float32)
    value = np.asarray(value, np.float32)
    Wq, Wk, Wv, Wo = (np.asarray(w, np.float32) for w in (Wq, Wk, Wv, Wo))
    in_maps = _host_prep(query, key, value, mask, Wq, Wk, Wv, Wo)
    if in_maps is None:  # non-causal mask: host fallback
        return _reference_fallback(query, key, value, mask, Wq, Wk, Wv, Wo, bo)
    nc = _get(1)
    res = run_bass_kernel_spmd(nc, in_maps, list(range(N_CORES)))
    return _gather(res.results, bo, query.shape[0])


def run_spmd(in_maps, repeat=1):
    """For test.py: run prebuilt kernel, return BassKernelResults."""
    nc = _get(repeat)
    return run_bass_kernel_spmd(nc, in_maps, list(range(N_CORES)))


def host_prep(*args, **kw):
    return _host_prep(*args, **kw)


def gather(results, bo, B=4):
    return _gather(results, bo, B)


# revision 40
# speedup vs baseline: 1.3843x; 1.3843x over previous
"""Multi-head causal attention on 8 Trainium2 NeuronCores (Bass/Tile).

Problem: B=4, S=1024, D=1024, H=16 heads (dk=64), causal mask, fp32 I/O.

Sharding: 8 cores = 4 batches x 2 head-groups (8 heads each).
  Wq/Wk/Wv sharded column-wise by head (tensor parallel), Wo row-wise;
  the Wo all-reduce is a host-side pairwise sum (2 cores per batch).

Per-core kernel (bf16 matmul operands, fp32 PSUM accumulate, ~4.7e-3
absmax-relative vs the fp32 reference):
  phase P: Q^T (zero-padded per head: slot h holds Q_h^T on its 64
           partitions, zeros elsewhere, so score matmuls contract K=128
           at full rate against the packed K^T without mixing heads),
           K^T packed [128, 4, S], V -> v_sb [128, ki, head, 65] with a
           65th ones column per head (softmax denominator trick).
           PSUM->SBUF copies run on the otherwise-idle ACT engine.
  phase A: per head-chunk hc and q-half qj: scores^T [k=128, q<=512]
           (causally width-trimmed), exp on ACT (no max subtraction:
           |scores/8| < ~6), 0/1 mask multiply only on the diagonal
           128-block, attnV accumulated over k-chunks with lhsT =
           V_ext [k, 65]; row 64 = denominator. Denominator rows are
           copied on ACT and DMA-gathered into den8 (keeps the DVE FIFO
           and PE stream free of per-pair round-trips).
  phase O: two batched reciprocals, selector-matmul broadcast of 1/den
           over partition halves, in-place normalize of headout^T,
           output projection accumulating over d-chunks, DMA out.
"""

from contextlib import ExitStack

import ml_dtypes
import numpy as np

import concourse.bacc as bacc
import concourse.tile as tile
from concourse import mybir
from concourse.bass_utils import run_bass_kernel_spmd

F32R = mybir.dt.float32r
F32 = mybir.dt.float32
BF16 = mybir.dt.bfloat16
EXP = mybir.ActivationFunctionType.Exp

S = 1024  # sequence length
D = 1024  # model dim
DK = 64  # head dim
HPC = 8  # heads per core
N_CORES = 8
SCALE = 1.0 / np.sqrt(DK)  # folded into the exp activation


def _emit(nc, tc, t, rep, phases=("P", "A", "O")):
    """Emit one full forward pass. `t` = dict of dram tensors."""
    ctx = ExitStack()
    with ctx:
        # ---- long-lived SBUF (per repeat; pools free at phase end) ----
        main = ctx.enter_context(tc.tile_pool(name=f"main{rep}", bufs=1))
        xpool = ctx.enter_context(tc.tile_pool(name=f"xin{rep}", bufs=2))
        wpool = ctx.enter_context(tc.tile_pool(name=f"win{rep}", bufs=2))

        # Q^T zero-padded per head: slot h holds Q_h^T on its 64 partitions,
        # zeros on the other 64 -> score matmuls contract K=128 (full rate)
        # against the packed kt_sb without mixing heads.
        qtz = main.tile([128, 8, S], BF16)
        kt_sb = main.tile([128, 4, S], BF16)
        v_sb = main.tile([128, 8, 8, 65], BF16)  # s-part: (ki, head, d+1)
        hout_sb = main.tile([128, 4, S], BF16)  # headout^T (unnormalized)
        maskd = main.tile([128, 128], BF16)  # diagonal-block 0/1 mask
        sel8 = main.tile([8, 512], F32R)
        den8 = main.tile([8, S], F32)
        rec8 = main.tile([8, S], F32R)
        wo_sb = main.tile([128, 4, S], BF16)

        nc.sync.dma_start(out=sel8, in_=t["sel8"][:, :])
        nc.sync.dma_start(out=maskd, in_=t["maskd"][:, :])
        nc.sync.dma_start(
            out=v_sb.rearrange("p a b c -> p (a b) c")[:, :, 64:65],
            in_=t["ones_col"][:, :, None],
        )
        nc.vector.memset(qtz.rearrange("p a b -> p (a b)"), 0.0)

        # ================= phase P: projections =================
        if "P" in phases:
         with (
            tc.tile_pool(name=f"pps{rep}", bufs=2, space="PSUM") as ppool,
        ):
            for which, xname, wname in (
                ("q", "xq_t", "wq_t"),
                ("k", "xk_t", "wk_t"),
            ):
                x_sb = xpool.tile([128, 8, S], BF16, tag="x")
                w_sb = wpool.tile([128, 8, 512], BF16, tag="w")
                xdr = t[xname].rearrange("(n p) s -> p n s", p=128)
                nc.sync.dma_start(
                    out=w_sb.rearrange("p a b -> p (a b)"), in_=t[wname][:, :]
                )
                for half in range(2):  # column halves: s-half chains start early
                    nc.sync.dma_start(
                        out=x_sb[:, :, 512 * half : 512 * (half + 1)],
                        in_=xdr[:, :, 512 * half : 512 * (half + 1)],
                    )
                for _ in (0,):
                    for sj in range(2):
                        for dtile in range(4):
                            ps = ppool.tile([128, 512], F32, tag="ps")
                            for c in range(8):
                                nc.tensor.matmul(
                                    ps,
                                    w_sb[:, c, 128 * dtile : 128 * (dtile + 1)],
                                    x_sb[:, c, 512 * sj : 512 * (sj + 1)],
                                    start=(c == 0),
                                    stop=(c == 7),
                                )
                            sjs = slice(512 * sj, 512 * (sj + 1))
                            ceng = (
                                nc.scalar.copy
                                if dtile % 2 == 0
                                else nc.vector.tensor_copy
                            )
                            if which == "q":
                                ceng(qtz[0:64, 2 * dtile, sjs], ps[0:64, :])
                                ceng(
                                    qtz[64:128, 2 * dtile + 1, sjs],
                                    ps[64:128, :],
                                )
                            else:
                                ceng(kt_sb[:, dtile, sjs], ps)

        # ========= phase A + O fused: per q-half, attention then outproj ====
        if "A" in phases:
         with (
            tc.tile_pool(name=f"avps{rep}", bufs=2, space="PSUM") as avpool,
            tc.tile_pool(name=f"epool{rep}", bufs=12) as epool,
            tc.tile_pool(name=f"xtr{rep}", bufs=6) as xtr,
            tc.tile_pool(name=f"osb{rep}", bufs=3) as osb,
        ):
            if "O" in phases:  # prefetch Wo during attention
                nc.sync.dma_start(
                    out=wo_sb.rearrange("p a b -> p (a b)"), in_=t["wo_s"][:, :]
                )
            # score pool scoped to attention only: closing it frees 4 PSUM
            # banks for the deeper (bufs=4) output-projection pool below
            scpool_cm = tc.tile_pool(name=f"scps{rep}", bufs=3, space="PSUM")
            scpool = scpool_cm.__enter__()
            # V projection shares the score pool so Q.K scores/exp overlap it
            xv_sb = xpool.tile([128, 8, S], BF16, tag="x")
            wv_sb = wpool.tile([128, 8, 512], BF16, tag="w")
            xvdr = t["xv_t"].rearrange("(n p) s -> p n s", p=128)
            nc.sync.dma_start(
                out=wv_sb.rearrange("p a b -> p (a b)"), in_=t["wv_t"][:, :]
            )
            for half in range(2):
                nc.sync.dma_start(
                    out=xv_sb[:, :, 512 * half : 512 * (half + 1)],
                    in_=xvdr[:, :, 512 * half : 512 * (half + 1)],
                )
            for tpair in range(4):
                ps2 = scpool.tile([128, 2, 512], F32, tag="sc")
                for sub in range(2):
                    stile = 2 * tpair + sub
                    for c in range(8):
                        nc.tensor.matmul(
                            ps2[:, sub, :],
                            xv_sb[:, c, 128 * stile : 128 * (stile + 1)],
                            wv_sb[:, c, :],
                            start=(c == 0),
                            stop=(c == 7),
                        )
                    vceng = (
                        nc.scalar.copy
                        if stile % 2 == 0
                        else nc.vector.tensor_copy
                    )
                    vceng(
                        v_sb[:, stile, :, 0:64],
                        ps2[:, sub, :].rearrange("p (h c) -> p h c", c=64),
                    )
            def emit_score(qj, hc, ki):
                b = 128 * max(0, ki - 4 * qj)
                kis = slice(128 * ki, 128 * (ki + 1))
                sc = scpool.tile([128, 2, 512], F32, tag="sc")
                nc.tensor.matmul(
                    sc[:, 0, b:512],
                    kt_sb[:, hc, kis],
                    qtz[:, 2 * hc, 512 * qj + b : 512 * (qj + 1)],
                    start=True,
                    stop=True,
                )
                nc.tensor.matmul(
                    sc[:, 1, b:512],
                    kt_sb[:, hc, kis],
                    qtz[:, 2 * hc + 1, 512 * qj + b : 512 * (qj + 1)],
                    start=True,
                    stop=True,
                )
                return sc

            steps = []
            for qj in range(2):
                kmax = 4 if qj == 0 else 8
                for hc in range(4):
                    for ki in range(kmax):
                        steps.append((qj, hc, ki, kmax))

            sc_next = emit_score(*steps[0][:3])
            avs = {}
            for i, (qj, hc, ki, kmax) in enumerate(steps):
                qsl = slice(512 * qj, 512 * (qj + 1))
                if ki == 0:
                    av_e = avpool.tile([128, 512], F32, tag="av")
                    av_o = avpool.tile([128, 512], F32, tag="av")
                    avs[(qj, hc)] = (av_e, av_o)
                o_e, o_o = avs[(qj, hc)]
                sc = sc_next
                if i + 1 < len(steps):  # cross-pair score lookahead
                    sc_next = emit_score(*steps[i + 1][:3])
                b = 128 * max(0, ki - 4 * qj)
                ee = epool.tile([128, 2, 512], BF16, tag="e")
                nc.scalar.activation(
                    ee[:, :, b:512],
                    sc[:, :, b:512],
                    EXP,
                    scale=float(SCALE),
                )
                if ki - 4 * qj >= 0:  # diagonal block: 0/1 mask
                    nc.vector.tensor_mul(
                        ee[:, 0, b : b + 128], ee[:, 0, b : b + 128], maskd
                    )
                    nc.vector.tensor_mul(
                        ee[:, 1, b : b + 128], ee[:, 1, b : b + 128], maskd
                    )
                nc.tensor.matmul(
                    o_e[0:65, b:512],
                    v_sb[:, ki, 2 * hc, :],
                    ee[:, 0, b:512],
                    start=(ki == 0),
                    stop=(ki == kmax - 1),
                )
                nc.tensor.matmul(
                    o_o[0:65, b:512],
                    v_sb[:, ki, 2 * hc + 1, :],
                    ee[:, 1, b:512],
                    start=(ki == 0),
                    stop=(ki == kmax - 1),
                )
                if ki != kmax - 1:
                    continue
                del avs[(qj, hc)]
                # extract headout^T + denominator rows
                nc.vector.tensor_copy(hout_sb[0:64, hc, qsl], o_e[0:64, :])
                otmp = xtr.tile([64, 512], BF16, tag="otmp")
                nc.vector.tensor_copy(otmp, o_o[0:64, :])
                nc.sync.dma_start(out=hout_sb[64:128, hc, qsl], in_=otmp)
                de_t = xtr.tile([1, 512], F32, tag="de")
                do_t = xtr.tile([1, 512], F32, tag="do")
                nc.vector.tensor_copy(de_t, o_e[64:65, :])
                nc.vector.tensor_copy(do_t, o_o[64:65, :])
                nc.sync.dma_start(out=den8[2 * hc : 2 * hc + 1, qsl], in_=de_t)
                nc.sync.dma_start(
                    out=den8[2 * hc + 1 : 2 * hc + 2, qsl], in_=do_t
                )
            scpool_cm.__exit__(None, None, None)
            opool_cm = tc.tile_pool(name=f"ops{rep}", bufs=2, space="PSUM")
            opool = opool_cm.__enter__()
            for qj in range(2) if "O" in phases else []:
                qsl = slice(512 * qj, 512 * (qj + 1))
                # ---- normalize + output projection for this q-half
                with nc.allow_low_precision(reason="softmax reciprocal"):
                    nc.vector.reciprocal(rec8[:, qsl], den8[:, qsl])
                for hc in range(4):
                    bp = opool.tile([128, 512], F32, tag="bp")
                    nc.tensor.matmul(
                        bp,
                        sel8[:, 128 * hc : 128 * (hc + 1)],
                        rec8[:, qsl],
                        start=True,
                        stop=True,
                    )
                    nc.vector.tensor_mul(
                        hout_sb[:, hc, qsl], hout_sb[:, hc, qsl], bp
                    )
                for stile in range(4 * qj, 4 * qj + 4):
                    out_sb = osb.tile([128, S], BF16, tag="out")
                    for ej in range(2):
                        fp = opool.tile([128, 512], F32, tag="op")
                        for hc in range(4):
                            nc.tensor.matmul(
                                fp,
                                hout_sb[:, hc, 128 * stile : 128 * (stile + 1)],
                                wo_sb[:, hc, 512 * ej : 512 * (ej + 1)],
                                start=(hc == 0),
                                stop=(hc == 3),
                            )
                        esl = slice(512 * ej, 512 * (ej + 1))
                        if ej == 0:
                            nc.vector.tensor_copy(out_sb[:, esl], fp)
                        else:
                            nc.scalar.copy(out_sb[:, esl], fp)
                    nc.sync.dma_start(
                        out=t["out_p"][128 * stile : 128 * (stile + 1), :],
                        in_=out_sb,
                    )
            opool_cm.__exit__(None, None, None)


def _build_phases(phases, repeat=1):
    return _build(repeat, phases=phases)


def _build(repeat=1, phases=("P", "A", "O")):
    nc = bacc.Bacc()
    t = {}
    for name in ("xq_t", "xk_t", "xv_t"):
        t[name] = nc.dram_tensor(name, [D, S], BF16, kind="ExternalInput")
    for name in ("wq_t", "wk_t", "wv_t"):
        t[name] = nc.dram_tensor(name, [128, 8 * 512], BF16, kind="ExternalInput")
    t["wo_s"] = nc.dram_tensor("wo_s", [128, 4 * D], BF16, kind="ExternalInput")
    t["maskd"] = nc.dram_tensor("maskd", [128, 128], BF16, kind="ExternalInput")
    t["sel8"] = nc.dram_tensor("sel8", [8, 512], F32R, kind="ExternalInput")
    t["ones_col"] = nc.dram_tensor("ones_col", [128, 64], BF16, kind="ExternalInput")
    t["out_p"] = nc.dram_tensor("out_p", [S, D], BF16, kind="ExternalOutput")

    with tile.TileContext(nc) as tc:
        if repeat == 1:
            _emit(nc, tc, t, 0, phases)
        else:
            with tc.For_i(0, repeat, 1):
                _emit(nc, tc, t, 0, phases)
    nc.compile()
    return nc


_CACHE = {}


def _get(repeat=1):
    if repeat not in _CACHE:
        _CACHE[repeat] = _build(repeat)
    return _CACHE[repeat]


def _host_prep(query, key, value, mask, Wq, Wk, Wv, Wo):
    """Build the per-core in_maps. Returns None if mask isn't causal tril."""
    m = np.asarray(mask)[0, 0]
    if not np.array_equal(m, np.tril(np.ones((S, S), m.dtype))):
        return None

    bf = ml_dtypes.bfloat16

    # diagonal-block mask (same for every diagonal tile under causal tril)
    maskd = m[0:128, 0:128].T.astype(bf)

    sel8 = np.zeros((8, 512), np.float32)
    for hc in range(4):
        sel8[2 * hc, 128 * hc : 128 * hc + 64] = 1.0
        sel8[2 * hc + 1, 128 * hc + 64 : 128 * hc + 128] = 1.0
    ones_col = np.ones((128, 64), bf)

    def ileave(a):  # [R, C] -> [128, (R//128)*C]: chunk-c data contiguous per p
        R, C = a.shape
        return np.ascontiguousarray(
            a.reshape(R // 128, 128, C).transpose(1, 0, 2).reshape(128, -1)
        )

    in_maps = []
    for c in range(N_CORES):
        b, g = c // 2, c % 2
        gsl = slice(512 * g, 512 * (g + 1))
        in_maps.append(
            {
                "xq_t": np.ascontiguousarray(query[b].T.astype(bf)),
                "xk_t": np.ascontiguousarray(key[b].T.astype(bf)),
                "xv_t": np.ascontiguousarray(value[b].T.astype(bf)),
                "wq_t": ileave(Wq[gsl, :].T.astype(bf)),
                "wk_t": ileave(Wk[gsl, :].T.astype(bf)),
                "wv_t": ileave(Wv[gsl, :].T.astype(bf)),
                "wo_s": ileave(Wo[:, gsl].T.astype(bf)),
                "maskd": maskd,
                "sel8": sel8,
                "ones_col": ones_col,
            }
        )
    return in_maps


def _gather(results, bo, B):
    out = np.empty((B, S, D), np.float32)
    for b in range(B):
        out[b] = (
            results[2 * b]["out_p"].astype(np.float32)
            + results[2 * b + 1]["out_p"].astype(np.float32)
            + np.asarray(bo)[None, :]
        )
    return out


def _reference_fallback(query, key, value, mask, Wq, Wk, Wv, Wo, bo):
    B = query.shape[0]
    H = 16
    dk = D // H
    q = np.asarray(query, np.float32)
    k = np.asarray(key, np.float32)
    v = np.asarray(value, np.float32)

    def proj(x, W):
        return (x @ W.T).reshape(B, S, H, dk).transpose(0, 2, 1, 3)

    Q, K, V = proj(q, Wq), proj(k, Wk), proj(v, Wv)
    sc = np.einsum("bhqd,bhkd->bhqk", Q, K) / np.sqrt(np.float32(dk))
    sc = np.where(np.asarray(mask) == 0, np.float32(-1e9), sc)
    sc = sc - sc.max(axis=-1, keepdims=True)
    a = np.exp(sc)
    a = a / a.sum(axis=-1, keepdims=True)
    o = np.einsum("bhqk,bhkd->bhqd", a, V).transpose(0, 2, 1, 3).reshape(B, S, D)
    return (o @ np.asarray(Wo).T + np.asarray(bo)).astype(np.float32)


def kernel(query, key, value, mask, Wq, Wk, Wv, Wo, bo):
    query = np.asarray(query, np.float32)
    key = np.asarray(key, np.float32)
    value = np.asarray(value, np.float32)
    Wq, Wk, Wv, Wo = (np.asarray(w, np.float32) for w in (Wq, Wk, Wv, Wo))
    in_maps = _host_prep(query, key, value, mask, Wq, Wk, Wv, Wo)
    if in_maps is None:  # non-causal mask: host fallback
        return _reference_fallback(query, key, value, mask, Wq, Wk, Wv, Wo, bo)
    nc = _get(1)
    res = run_bass_kernel_spmd(nc, in_maps, list(range(N_CORES)))
    return _gather(res.results, bo, query.shape[0])


def run_spmd(in_maps, repeat=1):
    """For test.py: run prebuilt kernel, return BassKernelResults."""
    nc = _get(repeat)
    return run_bass_kernel_spmd(nc, in_maps, list(range(N_CORES)))


def host_prep(*args, **kw):
    return _host_prep(*args, **kw)


def gather(results, bo, B=4):
    return _gather(results, bo, B)



# revision 41
# speedup vs baseline: 1.4484x; 1.0463x over previous
"""Multi-head causal attention on 8 Trainium2 NeuronCores (Bass/Tile).

Problem: B=4, S=1024, D=1024, H=16 heads (dk=64), causal mask, fp32 I/O.

Sharding: 8 cores = 4 batches x 2 head-groups (8 heads each).
  Wq/Wk/Wv sharded column-wise by head (tensor parallel), Wo row-wise;
  the Wo all-reduce is a host-side pairwise sum (2 cores per batch).

Per-core kernel (bf16 matmul operands, fp32 PSUM accumulate, ~4.7e-3
absmax-relative vs the fp32 reference):
  phase P: Q^T (zero-padded per head: slot h holds Q_h^T on its 64
           partitions, zeros elsewhere, so score matmuls contract K=128
           at full rate against the packed K^T without mixing heads),
           K^T packed [128, 4, S], V -> v_sb [128, ki, head, 65] with a
           65th ones column per head (softmax denominator trick).
           PSUM->SBUF copies run on the otherwise-idle ACT engine.
  phase A: per head-chunk hc and q-half qj: scores^T [k=128, q<=512]
           (causally width-trimmed), exp on ACT (no max subtraction:
           |scores/8| < ~6), 0/1 mask multiply only on the diagonal
           128-block, attnV accumulated over k-chunks with lhsT =
           V_ext [k, 65]; row 64 = denominator. Denominator rows are
           copied on ACT and DMA-gathered into den8 (keeps the DVE FIFO
           and PE stream free of per-pair round-trips).
  phase O: two batched reciprocals, selector-matmul broadcast of 1/den
           over partition halves, in-place normalize of headout^T,
           output projection accumulating over d-chunks, DMA out.
"""

from contextlib import ExitStack

import ml_dtypes
import numpy as np

import concourse.bacc as bacc
import concourse.tile as tile
from concourse import mybir
from concourse.bass_utils import run_bass_kernel_spmd

F32R = mybir.dt.float32r
F32 = mybir.dt.float32
BF16 = mybir.dt.bfloat16
EXP = mybir.ActivationFunctionType.Exp

S = 1024  # sequence length
D = 1024  # model dim
DK = 64  # head dim
HPC = 8  # heads per core
N_CORES = 8
SCALE = 1.0 / np.sqrt(DK)  # folded into the exp activation


def _emit(nc, tc, t, rep, phases=("P", "A", "O"), ldeng=None):
    ldeng = ldeng or nc.sync
    """Emit one full forward pass. `t` = dict of dram tensors."""
    ctx = ExitStack()
    with ctx:
        # ---- long-lived SBUF (per repeat; pools free at phase end) ----
        main = ctx.enter_context(tc.tile_pool(name=f"main{rep}", bufs=1))
        xpool = ctx.enter_context(tc.tile_pool(name=f"xin{rep}", bufs=2))
        wpool = ctx.enter_context(tc.tile_pool(name=f"win{rep}", bufs=2))

        # Q^T zero-padded per head: slot h holds Q_h^T on its 64 partitions,
        # zeros on the other 64 -> score matmuls contract K=128 (full rate)
        # against the packed kt_sb without mixing heads.
        qtz = main.tile([128, 8, S], BF16)
        kt_sb = main.tile([128, 4, S], BF16)
        v_sb = main.tile([128, 8, 8, 65], BF16)  # s-part: (ki, head, d+1)
        hout_sb = main.tile([128, 4, S], BF16)  # headout^T (unnormalized)
        maskd = main.tile([128, 128], BF16)  # diagonal-block 0/1 mask
        sel8 = main.tile([8, 512], F32R)
        den8 = main.tile([8, S], F32)
        rec8 = main.tile([8, S], F32R)
        wo_sb = main.tile([128, 4, S], BF16)

        ldeng.dma_start(out=sel8, in_=t["sel8"][:, :])
        ldeng.dma_start(out=maskd, in_=t["maskd"][:, :])
        ldeng.dma_start(
            out=v_sb.rearrange("p a b c -> p (a b) c")[:, :, 64:65],
            in_=t["ones_col"][:, :, None],
        )
        nc.vector.memset(qtz.rearrange("p a b -> p (a b)"), 0.0)

        # ================= phase P: projections =================
        if "P" in phases:
         with (
            tc.tile_pool(name=f"pps{rep}", bufs=2, space="PSUM") as ppool,
        ):
            for which, xname, wname in (
                ("q", "xq_t", "wq_t"),
                ("k", "xk_t", "wk_t"),
            ):
                x_sb = xpool.tile([128, 8, S], BF16, tag="x")
                w_sb = wpool.tile([128, 8, 512], BF16, tag="w")
                xdr = t[xname].rearrange("(n p) s -> p n s", p=128)
                ldeng.dma_start(
                    out=w_sb.rearrange("p a b -> p (a b)"), in_=t[wname][:, :]
                )
                for half in range(2):  # column halves: s-half chains start early
                    ldeng.dma_start(
                        out=x_sb[:, :, 512 * half : 512 * (half + 1)],
                        in_=xdr[:, :, 512 * half : 512 * (half + 1)],
                    )
                for _ in (0,):
                    for sj in range(2):
                        for dtile in range(4):
                            ps = ppool.tile([128, 512], F32, tag="ps")
                            for c in range(8):
                                nc.tensor.matmul(
                                    ps,
                                    w_sb[:, c, 128 * dtile : 128 * (dtile + 1)],
                                    x_sb[:, c, 512 * sj : 512 * (sj + 1)],
                                    start=(c == 0),
                                    stop=(c == 7),
                                )
                            sjs = slice(512 * sj, 512 * (sj + 1))
                            ceng = (
                                nc.scalar.copy
                                if dtile % 2 == 0
                                else nc.vector.tensor_copy
                            )
                            if which == "q":
                                ceng(qtz[0:64, 2 * dtile, sjs], ps[0:64, :])
                                ceng(
                                    qtz[64:128, 2 * dtile + 1, sjs],
                                    ps[64:128, :],
                                )
                            else:
                                ceng(kt_sb[:, dtile, sjs], ps)

        # ========= phase A + O fused: per q-half, attention then outproj ====
        if "A" in phases:
         with (
            tc.tile_pool(name=f"avps{rep}", bufs=2, space="PSUM") as avpool,
            tc.tile_pool(name=f"epool{rep}", bufs=12) as epool,
            tc.tile_pool(name=f"xtr{rep}", bufs=6) as xtr,
            tc.tile_pool(name=f"osb{rep}", bufs=3) as osb,
        ):
            if "O" in phases:  # prefetch Wo during attention
                ldeng.dma_start(
                    out=wo_sb.rearrange("p a b -> p (a b)"), in_=t["wo_s"][:, :]
                )
            # score pool scoped to attention only: closing it frees 4 PSUM
            # banks for the deeper (bufs=4) output-projection pool below
            scpool_cm = tc.tile_pool(name=f"scps{rep}", bufs=3, space="PSUM")
            scpool = scpool_cm.__enter__()
            # V projection shares the score pool so Q.K scores/exp overlap it
            xv_sb = xpool.tile([128, 8, S], BF16, tag="x")
            wv_sb = wpool.tile([128, 8, 512], BF16, tag="w")
            xvdr = t["xv_t"].rearrange("(n p) s -> p n s", p=128)
            ldeng.dma_start(
                out=wv_sb.rearrange("p a b -> p (a b)"), in_=t["wv_t"][:, :]
            )
            for half in range(2):
                ldeng.dma_start(
                    out=xv_sb[:, :, 512 * half : 512 * (half + 1)],
                    in_=xvdr[:, :, 512 * half : 512 * (half + 1)],
                )
            for tpair in range(4):
                ps2 = scpool.tile([128, 2, 512], F32, tag="sc")
                for sub in range(2):
                    stile = 2 * tpair + sub
                    for c in range(8):
                        nc.tensor.matmul(
                            ps2[:, sub, :],
                            xv_sb[:, c, 128 * stile : 128 * (stile + 1)],
                            wv_sb[:, c, :],
                            start=(c == 0),
                            stop=(c == 7),
                        )
                    vceng = (
                        nc.scalar.copy
                        if stile % 2 == 0
                        else nc.vector.tensor_copy
                    )
                    vceng(
                        v_sb[:, stile, :, 0:64],
                        ps2[:, sub, :].rearrange("p (h c) -> p h c", c=64),
                    )
            def emit_score(qj, hc, ki):
                b = 128 * max(0, ki - 4 * qj)
                kis = slice(128 * ki, 128 * (ki + 1))
                sc = scpool.tile([128, 2, 512], F32, tag="sc")
                nc.tensor.matmul(
                    sc[:, 0, b:512],
                    kt_sb[:, hc, kis],
                    qtz[:, 2 * hc, 512 * qj + b : 512 * (qj + 1)],
                    start=True,
                    stop=True,
                )
                nc.tensor.matmul(
                    sc[:, 1, b:512],
                    kt_sb[:, hc, kis],
                    qtz[:, 2 * hc + 1, 512 * qj + b : 512 * (qj + 1)],
                    start=True,
                    stop=True,
                )
                return sc

            steps = []
            for qj in range(2):
                kmax = 4 if qj == 0 else 8
                for hc in range(4):
                    for ki in range(kmax):
                        steps.append((qj, hc, ki, kmax))

            sc_next = emit_score(*steps[0][:3])
            avs = {}
            for i, (qj, hc, ki, kmax) in enumerate(steps):
                qsl = slice(512 * qj, 512 * (qj + 1))
                if ki == 0:
                    av_e = avpool.tile([128, 512], F32, tag="av")
                    av_o = avpool.tile([128, 512], F32, tag="av")
                    avs[(qj, hc)] = (av_e, av_o)
                o_e, o_o = avs[(qj, hc)]
                sc = sc_next
                if i + 1 < len(steps):  # cross-pair score lookahead
                    sc_next = emit_score(*steps[i + 1][:3])
                b = 128 * max(0, ki - 4 * qj)
                ee = epool.tile([128, 2, 512], BF16, tag="e")
                nc.scalar.activation(
                    ee[:, :, b:512],
                    sc[:, :, b:512],
                    EXP,
                    scale=float(SCALE),
                )
                if ki - 4 * qj >= 0:  # diagonal block: 0/1 mask
                    nc.vector.tensor_mul(
                        ee[:, 0, b : b + 128], ee[:, 0, b : b + 128], maskd
                    )
                    nc.vector.tensor_mul(
                        ee[:, 1, b : b + 128], ee[:, 1, b : b + 128], maskd
                    )
                nc.tensor.matmul(
                    o_e[0:65, b:512],
                    v_sb[:, ki, 2 * hc, :],
                    ee[:, 0, b:512],
                    start=(ki == 0),
                    stop=(ki == kmax - 1),
                )
                nc.tensor.matmul(
                    o_o[0:65, b:512],
                    v_sb[:, ki, 2 * hc + 1, :],
                    ee[:, 1, b:512],
                    start=(ki == 0),
                    stop=(ki == kmax - 1),
                )
                if ki != kmax - 1:
                    continue
                del avs[(qj, hc)]
                # extract headout^T + denominator rows
                nc.vector.tensor_copy(hout_sb[0:64, hc, qsl], o_e[0:64, :])
                otmp = xtr.tile([64, 512], BF16, tag="otmp")
                nc.vector.tensor_copy(otmp, o_o[0:64, :])
                nc.sync.dma_start(out=hout_sb[64:128, hc, qsl], in_=otmp)
                de_t = xtr.tile([1, 512], F32, tag="de")
                do_t = xtr.tile([1, 512], F32, tag="do")
                nc.vector.tensor_copy(de_t, o_e[64:65, :])
                nc.vector.tensor_copy(do_t, o_o[64:65, :])
                nc.sync.dma_start(out=den8[2 * hc : 2 * hc + 1, qsl], in_=de_t)
                nc.sync.dma_start(
                    out=den8[2 * hc + 1 : 2 * hc + 2, qsl], in_=do_t
                )
            scpool_cm.__exit__(None, None, None)
            opool_cm = tc.tile_pool(name=f"ops{rep}", bufs=2, space="PSUM")
            opool = opool_cm.__enter__()
            for qj in range(2) if "O" in phases else []:
                qsl = slice(512 * qj, 512 * (qj + 1))
                # ---- normalize + output projection for this q-half
                with nc.allow_low_precision(reason="softmax reciprocal"):
                    nc.vector.reciprocal(rec8[:, qsl], den8[:, qsl])
                for hc in range(4):
                    bp = opool.tile([128, 512], F32, tag="bp")
                    nc.tensor.matmul(
                        bp,
                        sel8[:, 128 * hc : 128 * (hc + 1)],
                        rec8[:, qsl],
                        start=True,
                        stop=True,
                    )
                    nc.vector.tensor_mul(
                        hout_sb[:, hc, qsl], hout_sb[:, hc, qsl], bp
                    )
                for stile in range(4 * qj, 4 * qj + 4):
                    out_sb = osb.tile([128, S], BF16, tag="out")
                    for ej in range(2):
                        fp = opool.tile([128, 512], F32, tag="op")
                        for hc in range(4):
                            nc.tensor.matmul(
                                fp,
                                hout_sb[:, hc, 128 * stile : 128 * (stile + 1)],
                                wo_sb[:, hc, 512 * ej : 512 * (ej + 1)],
                                start=(hc == 0),
                                stop=(hc == 3),
                            )
                        esl = slice(512 * ej, 512 * (ej + 1))
                        if ej == 0:
                            nc.vector.tensor_copy(out_sb[:, esl], fp)
                        else:
                            nc.scalar.copy(out_sb[:, esl], fp)
                    nc.sync.dma_start(
                        out=t["out_p"][128 * stile : 128 * (stile + 1), :],
                        in_=out_sb,
                    )
            opool_cm.__exit__(None, None, None)


def _build_phases(phases, repeat=1):
    return _build(repeat, phases=phases)


def _build(repeat=1, phases=("P", "A", "O")):
    nc = bacc.Bacc()
    t = {}
    for name in ("xq_t", "xk_t", "xv_t"):
        t[name] = nc.dram_tensor(name, [D, S], BF16, kind="ExternalInput")
    for name in ("wq_t", "wk_t", "wv_t"):
        t[name] = nc.dram_tensor(name, [128, 8 * 512], BF16, kind="ExternalInput")
    t["wo_s"] = nc.dram_tensor("wo_s", [128, 4 * D], BF16, kind="ExternalInput")
    t["maskd"] = nc.dram_tensor("maskd", [128, 128], BF16, kind="ExternalInput")
    t["sel8"] = nc.dram_tensor("sel8", [8, 512], F32R, kind="ExternalInput")
    t["ones_col"] = nc.dram_tensor("ones_col", [128, 64], BF16, kind="ExternalInput")
    t["out_p"] = nc.dram_tensor("out_p", [S, D], BF16, kind="ExternalOutput")

    with tile.TileContext(nc) as tc:
        if repeat == 1:
            _emit(nc, tc, t, 0, phases)
        else:
            n2, rem = divmod(repeat, 2)
            if n2:
                with tc.For_i(0, n2, 1):
                    _emit(nc, tc, t, 0, phases, ldeng=nc.sync)
                    _emit(nc, tc, t, 1, phases, ldeng=nc.scalar)
            for j in range(rem):
                _emit(nc, tc, t, 2 + j, phases)
    nc.compile()
    return nc


_CACHE = {}


def _get(repeat=1):
    if repeat not in _CACHE:
        _CACHE[repeat] = _build(repeat)
    return _CACHE[repeat]


def _host_prep(query, key, value, mask, Wq, Wk, Wv, Wo):
    """Build the per-core in_maps. Returns None if mask isn't causal tril."""
    m = np.asarray(mask)[0, 0]
    if not np.array_equal(m, np.tril(np.ones((S, S), m.dtype))):
        return None

    bf = ml_dtypes.bfloat16

    # diagonal-block mask (same for every diagonal tile under causal tril)
    maskd = m[0:128, 0:128].T.astype(bf)

    sel8 = np.zeros((8, 512), np.float32)
    for hc in range(4):
        sel8[2 * hc, 128 * hc : 128 * hc + 64] = 1.0
        sel8[2 * hc + 1, 128 * hc + 64 : 128 * hc + 128] = 1.0
    ones_col = np.ones((128, 64), bf)

    def ileave(a):  # [R, C] -> [128, (R//128)*C]: chunk-c data contiguous per p
        R, C = a.shape
        return np.ascontiguousarray(
            a.reshape(R // 128, 128, C).transpose(1, 0, 2).reshape(128, -1)
        )

    in_maps = []
    for c in range(N_CORES):
        b, g = c // 2, c % 2
        gsl = slice(512 * g, 512 * (g + 1))
        in_maps.append(
            {
                "xq_t": np.ascontiguousarray(query[b].T.astype(bf)),
                "xk_t": np.ascontiguousarray(key[b].T.astype(bf)),
                "xv_t": np.ascontiguousarray(value[b].T.astype(bf)),
                "wq_t": ileave(Wq[gsl, :].T.astype(bf)),
                "wk_t": ileave(Wk[gsl, :].T.astype(bf)),
                "wv_t": ileave(Wv[gsl, :].T.astype(bf)),
                "wo_s": ileave(Wo[:, gsl].T.astype(bf)),
                "maskd": maskd,
                "sel8": sel8,
                "ones_col": ones_col,
            }
        )
    return in_maps


def _gather(results, bo, B):
    out = np.empty((B, S, D), np.float32)
    for b in range(B):
        out[b] = (
            results[2 * b]["out_p"].astype(np.float32)
            + results[2 * b + 1]["out_p"].astype(np.float32)
            + np.asarray(bo)[None, :]
        )
    return out


def _reference_fallback(query, key, value, mask, Wq, Wk, Wv, Wo, bo):
    B = query.shape[0]
    H = 16
    dk = D // H
    q = np.asarray(query, np.float32)
    k = np.asarray(key, np.float32)
    v = np.asarray(value, np.float32)

    def proj(x, W):
        return (x @ W.T).reshape(B, S, H, dk).transpose(0, 2, 1, 3)

    Q, K, V = proj(q, Wq), proj(k, Wk), proj(v, Wv)
    sc = np.einsum("bhqd,bhkd->bhqk", Q, K) / np.sqrt(np.float32(dk))
    sc = np.where(np.asarray(mask) == 0, np.float32(-1e9), sc)
    sc = sc - sc.max(axis=-1, keepdims=True)
    a = np.exp(sc)
    a = a / a.sum(axis=-1, keepdims=True)
    o = np.einsum("bhqk,bhkd->bhqd", a, V).transpose(0, 2, 1, 3).reshape(B, S, D)
    return (o @ np.asarray(Wo).T + np.asarray(bo)).astype(np.float32)


def kernel(query, key, value, mask, Wq, Wk, Wv, Wo, bo):
    query = np.asarray(query, np.float32)
    key = np.asarray(key, np.float32)
    value = np.asarray(value, np.float32)
    Wq, Wk, Wv, Wo = (np.asarray(w, np.float32) for w in (Wq, Wk, Wv, Wo))
    in_maps = _host_prep(query, key, value, mask, Wq, Wk, Wv, Wo)
    if in_maps is None:  # non-causal mask: host fallback
        return _reference_fallback(query, key, value, mask, Wq, Wk, Wv, Wo, bo)
    nc = _get(1)
    res = run_bass_kernel_spmd(nc, in_maps, list(range(N_CORES)))
    return _gather(res.results, bo, query.shape[0])


def run_spmd(in_maps, repeat=1):
    """For test.py: run prebuilt kernel, return BassKernelResults."""
    nc = _get(repeat)
    return run_bass_kernel_spmd(nc, in_maps, list(range(N_CORES)))


def host_prep(*args, **kw):
    return _host_prep(*args, **kw)


def gather(results, bo, B=4):
    return _gather(results, bo, B)



# revision 43
# speedup vs baseline: 1.5264x; 1.0538x over previous
"""Multi-head causal attention on 8 Trainium2 NeuronCores (Bass/Tile).

Problem: B=4, S=1024, D=1024, H=16 heads (dk=64), causal mask, fp32 I/O.

Sharding: 8 cores = 4 batches x 2 head-groups (8 heads each).
  Wq/Wk/Wv sharded column-wise by head (tensor parallel), Wo row-wise;
  the Wo all-reduce is a host-side pairwise sum (2 cores per batch).

Per-core kernel (bf16 matmul operands, fp32 PSUM accumulate, ~4.7e-3
absmax-relative vs the fp32 reference):
  phase P: Q^T (zero-padded per head: slot h holds Q_h^T on its 64
           partitions, zeros elsewhere, so score matmuls contract K=128
           at full rate against the packed K^T without mixing heads),
           K^T packed [128, 4, S], V -> v_sb [128, ki, head, 65] with a
           65th ones column per head (softmax denominator trick).
           PSUM->SBUF copies run on the otherwise-idle ACT engine.
  phase A: per head-chunk hc and q-half qj: scores^T [k=128, q<=512]
           (causally width-trimmed), exp on ACT (no max subtraction:
           |scores/8| < ~6), 0/1 mask multiply only on the diagonal
           128-block, attnV accumulated over k-chunks with lhsT =
           V_ext [k, 65]; row 64 = denominator. Denominator rows are
           copied on ACT and DMA-gathered into den8 (keeps the DVE FIFO
           and PE stream free of per-pair round-trips).
  phase O: two batched reciprocals, selector-matmul broadcast of 1/den
           over partition halves, in-place normalize of headout^T,
           output projection accumulating over d-chunks, DMA out.
"""

from contextlib import ExitStack

import ml_dtypes
import numpy as np

import concourse.bacc as bacc
import concourse.tile as tile
from concourse import mybir
from concourse.bass_utils import run_bass_kernel_spmd

F32R = mybir.dt.float32r
F32 = mybir.dt.float32
BF16 = mybir.dt.bfloat16
EXP = mybir.ActivationFunctionType.Exp

S = 1024  # sequence length
D = 1024  # model dim
DK = 64  # head dim
HPC = 8  # heads per core
N_CORES = 8
SCALE = 1.0 / np.sqrt(DK)  # folded into the exp activation


def _emit(nc, tc, t, rep, phases=("P", "A", "O"), ldeng=None, steng=None):
    ldeng = ldeng or nc.sync
    steng = steng or nc.sync
    """Emit one full forward pass. `t` = dict of dram tensors."""
    ctx = ExitStack()
    with ctx:
        # ---- long-lived SBUF (per repeat; pools free at phase end) ----
        main = ctx.enter_context(tc.tile_pool(name=f"main{rep}", bufs=1))
        xpool = ctx.enter_context(tc.tile_pool(name=f"xin{rep}", bufs=2))
        wpool = ctx.enter_context(tc.tile_pool(name=f"win{rep}", bufs=2))

        # Q^T zero-padded per head: slot h holds Q_h^T on its 64 partitions,
        # zeros on the other 64 -> score matmuls contract K=128 (full rate)
        # against the packed kt_sb without mixing heads.
        qtz = main.tile([128, 8, S], BF16)
        kt_sb = main.tile([128, 4, S], BF16)
        v_sb = main.tile([128, 8, 8, 65], BF16)  # s-part: (ki, head, d+1)
        hout_sb = main.tile([128, 4, S], BF16)  # headout^T (unnormalized)
        maskd = main.tile([128, 128], BF16)  # diagonal-block 0/1 mask
        sel8 = main.tile([8, 512], F32R)
        den8 = main.tile([8, S], F32)
        rec8 = main.tile([8, S], F32R)
        wo_sb = main.tile([128, 4, S], BF16)

        ldeng.dma_start(out=sel8, in_=t["sel8"][:, :])
        ldeng.dma_start(out=maskd, in_=t["maskd"][:, :])
        ldeng.dma_start(
            out=v_sb.rearrange("p a b c -> p (a b) c")[:, :, 64:65],
            in_=t["ones_col"][:, :, None],
        )
        nc.gpsimd.memset(qtz.rearrange("p a b -> p (a b)"), 0.0)

        # ================= phase P: projections =================
        if "P" in phases:
         with (
            tc.tile_pool(name=f"pps{rep}", bufs=2, space="PSUM") as ppool,
        ):
            for which, xname, wname in (
                ("q", "xq_t", "wq_t"),
                ("k", "xk_t", "wk_t"),
            ):
                x_sb = xpool.tile([128, 8, S], BF16, tag="x")
                w_sb = wpool.tile([128, 8, 512], BF16, tag="w")
                xdr = t[xname].rearrange("(n p) s -> p n s", p=128)
                ldeng.dma_start(
                    out=w_sb.rearrange("p a b -> p (a b)"), in_=t[wname][:, :]
                )
                for half in range(2):  # column halves: s-half chains start early
                    ldeng.dma_start(
                        out=x_sb[:, :, 512 * half : 512 * (half + 1)],
                        in_=xdr[:, :, 512 * half : 512 * (half + 1)],
                    )
                for _ in (0,):
                    for sj in range(2):
                        for dtile in range(4):
                            ps = ppool.tile([128, 512], F32, tag="ps")
                            for c in range(8):
                                nc.tensor.matmul(
                                    ps,
                                    w_sb[:, c, 128 * dtile : 128 * (dtile + 1)],
                                    x_sb[:, c, 512 * sj : 512 * (sj + 1)],
                                    start=(c == 0),
                                    stop=(c == 7),
                                )
                            sjs = slice(512 * sj, 512 * (sj + 1))
                            ceng = (
                                nc.scalar.copy
                                if dtile % 2 == 0
                                else nc.vector.tensor_copy
                            )
                            if which == "q":
                                ceng(qtz[0:64, 2 * dtile, sjs], ps[0:64, :])
                                ceng(
                                    qtz[64:128, 2 * dtile + 1, sjs],
                                    ps[64:128, :],
                                )
                            else:
                                ceng(kt_sb[:, dtile, sjs], ps)

        # ========= phase A + O fused: per q-half, attention then outproj ====
        if "A" in phases:
         with (
            tc.tile_pool(name=f"avps{rep}", bufs=2, space="PSUM") as avpool,
            tc.tile_pool(name=f"epool{rep}", bufs=12) as epool,
            tc.tile_pool(name=f"xtr{rep}", bufs=6) as xtr,
            tc.tile_pool(name=f"osb{rep}", bufs=3) as osb,
        ):
            if "O" in phases:  # prefetch Wo during attention
                ldeng.dma_start(
                    out=wo_sb.rearrange("p a b -> p (a b)"), in_=t["wo_s"][:, :]
                )
            # score pool scoped to attention only: closing it frees 4 PSUM
            # banks for the deeper (bufs=4) output-projection pool below
            scpool_cm = tc.tile_pool(name=f"scps{rep}", bufs=3, space="PSUM")
            scpool = scpool_cm.__enter__()
            # V projection shares the score pool so Q.K scores/exp overlap it
            xv_sb = xpool.tile([128, 8, S], BF16, tag="x")
            wv_sb = wpool.tile([128, 8, 512], BF16, tag="w")
            xvdr = t["xv_t"].rearrange("(n p) s -> p n s", p=128)
            ldeng.dma_start(
                out=wv_sb.rearrange("p a b -> p (a b)"), in_=t["wv_t"][:, :]
            )
            for half in range(2):
                ldeng.dma_start(
                    out=xv_sb[:, :, 512 * half : 512 * (half + 1)],
                    in_=xvdr[:, :, 512 * half : 512 * (half + 1)],
                )
            for tpair in range(4):
                ps2 = scpool.tile([128, 2, 512], F32, tag="sc")
                for sub in range(2):
                    stile = 2 * tpair + sub
                    for c in range(8):
                        nc.tensor.matmul(
                            ps2[:, sub, :],
                            xv_sb[:, c, 128 * stile : 128 * (stile + 1)],
                            wv_sb[:, c, :],
                            start=(c == 0),
                            stop=(c == 7),
                        )
                    vceng = (
                        nc.scalar.copy
                        if stile % 2 == 0
                        else nc.vector.tensor_copy
                    )
                    vceng(
                        v_sb[:, stile, :, 0:64],
                        ps2[:, sub, :].rearrange("p (h c) -> p h c", c=64),
                    )
            def emit_score(qj, hc, ki):
                b = 128 * max(0, ki - 4 * qj)
                kis = slice(128 * ki, 128 * (ki + 1))
                sc = scpool.tile([128, 2, 512], F32, tag="sc")
                nc.tensor.matmul(
                    sc[:, 0, b:512],
                    kt_sb[:, hc, kis],
                    qtz[:, 2 * hc, 512 * qj + b : 512 * (qj + 1)],
                    start=True,
                    stop=True,
                )
                nc.tensor.matmul(
                    sc[:, 1, b:512],
                    kt_sb[:, hc, kis],
                    qtz[:, 2 * hc + 1, 512 * qj + b : 512 * (qj + 1)],
                    start=True,
                    stop=True,
                )
                return sc

            steps = []
            for qj in range(2):
                kmax = 4 if qj == 0 else 8
                for hc in range(4):
                    for ki in range(kmax):
                        steps.append((qj, hc, ki, kmax))

            sc_next = emit_score(*steps[0][:3])
            avs = {}
            for i, (qj, hc, ki, kmax) in enumerate(steps):
                qsl = slice(512 * qj, 512 * (qj + 1))
                if ki == 0:
                    av_e = avpool.tile([128, 512], F32, tag="av")
                    av_o = avpool.tile([128, 512], F32, tag="av")
                    avs[(qj, hc)] = (av_e, av_o)
                o_e, o_o = avs[(qj, hc)]
                sc = sc_next
                if i + 1 < len(steps):  # cross-pair score lookahead
                    sc_next = emit_score(*steps[i + 1][:3])
                b = 128 * max(0, ki - 4 * qj)
                ee = epool.tile([128, 2, 512], BF16, tag="e")
                nc.scalar.activation(
                    ee[:, :, b:512],
                    sc[:, :, b:512],
                    EXP,
                    scale=float(SCALE),
                )
                if ki - 4 * qj >= 0:  # diagonal block: 0/1 mask
                    nc.vector.tensor_mul(
                        ee[:, 0, b : b + 128], ee[:, 0, b : b + 128], maskd
                    )
                    nc.vector.tensor_mul(
                        ee[:, 1, b : b + 128], ee[:, 1, b : b + 128], maskd
                    )
                nc.tensor.matmul(
                    o_e[0:65, b:512],
                    v_sb[:, ki, 2 * hc, :],
                    ee[:, 0, b:512],
                    start=(ki == 0),
                    stop=(ki == kmax - 1),
                )
                nc.tensor.matmul(
                    o_o[0:65, b:512],
                    v_sb[:, ki, 2 * hc + 1, :],
                    ee[:, 1, b:512],
                    start=(ki == 0),
                    stop=(ki == kmax - 1),
                )
                if ki != kmax - 1:
                    continue
                del avs[(qj, hc)]
                # extract headout^T + denominator rows
                nc.vector.tensor_copy(hout_sb[0:64, hc, qsl], o_e[0:64, :])
                otmp = xtr.tile([64, 512], BF16, tag="otmp")
                nc.vector.tensor_copy(otmp, o_o[0:64, :])
                nc.sync.dma_start(out=hout_sb[64:128, hc, qsl], in_=otmp)
                de_t = xtr.tile([1, 512], F32, tag="de")
                do_t = xtr.tile([1, 512], F32, tag="do")
                nc.vector.tensor_copy(de_t, o_e[64:65, :])
                nc.vector.tensor_copy(do_t, o_o[64:65, :])
                nc.sync.dma_start(out=den8[2 * hc : 2 * hc + 1, qsl], in_=de_t)
                nc.sync.dma_start(
                    out=den8[2 * hc + 1 : 2 * hc + 2, qsl], in_=do_t
                )
            scpool_cm.__exit__(None, None, None)
            opool_cm = tc.tile_pool(name=f"ops{rep}", bufs=2, space="PSUM")
            opool = opool_cm.__enter__()
            for qj in range(2) if "O" in phases else []:
                qsl = slice(512 * qj, 512 * (qj + 1))
                # ---- normalize + output projection for this q-half
                with nc.allow_low_precision(reason="softmax reciprocal"):
                    nc.vector.reciprocal(rec8[:, qsl], den8[:, qsl])
                for hc in range(4):
                    bp = opool.tile([128, 512], F32, tag="bp")
                    nc.tensor.matmul(
                        bp,
                        sel8[:, 128 * hc : 128 * (hc + 1)],
                        rec8[:, qsl],
                        start=True,
                        stop=True,
                    )
                    nc.vector.tensor_mul(
                        hout_sb[:, hc, qsl], hout_sb[:, hc, qsl], bp
                    )
                for stile in range(4 * qj, 4 * qj + 4):
                    out_sb = osb.tile([128, S], BF16, tag="out")
                    for ej in range(2):
                        fp = opool.tile([128, 512], F32, tag="op")
                        for hc in range(4):
                            nc.tensor.matmul(
                                fp,
                                hout_sb[:, hc, 128 * stile : 128 * (stile + 1)],
                                wo_sb[:, hc, 512 * ej : 512 * (ej + 1)],
                                start=(hc == 0),
                                stop=(hc == 3),
                            )
                        esl = slice(512 * ej, 512 * (ej + 1))
                        if ej == 0:
                            nc.vector.tensor_copy(out_sb[:, esl], fp)
                        else:
                            nc.scalar.copy(out_sb[:, esl], fp)
                    steng.dma_start(
                        out=t["out_p"][128 * stile : 128 * (stile + 1), :],
                        in_=out_sb,
                    )
            opool_cm.__exit__(None, None, None)


def _build_phases(phases, repeat=1):
    return _build(repeat, phases=phases)


def _build(repeat=1, phases=("P", "A", "O")):
    nc = bacc.Bacc()
    t = {}
    for name in ("xq_t", "xk_t", "xv_t"):
        t[name] = nc.dram_tensor(name, [D, S], BF16, kind="ExternalInput")
    for name in ("wq_t", "wk_t", "wv_t"):
        t[name] = nc.dram_tensor(name, [128, 8 * 512], BF16, kind="ExternalInput")
    t["wo_s"] = nc.dram_tensor("wo_s", [128, 4 * D], BF16, kind="ExternalInput")
    t["maskd"] = nc.dram_tensor("maskd", [128, 128], BF16, kind="ExternalInput")
    t["sel8"] = nc.dram_tensor("sel8", [8, 512], F32R, kind="ExternalInput")
    t["ones_col"] = nc.dram_tensor("ones_col", [128, 64], BF16, kind="ExternalInput")
    t["out_p"] = nc.dram_tensor("out_p", [S, D], BF16, kind="ExternalOutput")

    with tile.TileContext(nc) as tc:
        if repeat == 1:
            _emit(nc, tc, t, 0, phases)
        else:
            n3, rem = divmod(repeat, 3)
            if n3:
                with tc.For_i(0, n3, 1):
                    _emit(nc, tc, t, 0, phases, ldeng=nc.sync)
                    _emit(nc, tc, t, 1, phases, ldeng=nc.scalar,
                          steng=nc.scalar)
                    _emit(nc, tc, t, 2, phases, ldeng=nc.sync)
            for j in range(rem):
                _emit(nc, tc, t, 3 + j, phases)
    nc.compile()
    return nc


_CACHE = {}


def _get(repeat=1):
    if repeat not in _CACHE:
        _CACHE[repeat] = _build(repeat)
    return _CACHE[repeat]


def _host_prep(query, key, value, mask, Wq, Wk, Wv, Wo):
    """Build the per-core in_maps. Returns None if mask isn't causal tril."""
    m = np.asarray(mask)[0, 0]
    if not np.array_equal(m, np.tril(np.ones((S, S), m.dtype))):
        return None

    bf = ml_dtypes.bfloat16

    # diagonal-block mask (same for every diagonal tile under causal tril)
    maskd = m[0:128, 0:128].T.astype(bf)

    sel8 = np.zeros((8, 512), np.float32)
    for hc in range(4):
        sel8[2 * hc, 128 * hc : 128 * hc + 64] = 1.0
        sel8[2 * hc + 1, 128 * hc + 64 : 128 * hc + 128] = 1.0
    ones_col = np.ones((128, 64), bf)

    def ileave(a):  # [R, C] -> [128, (R//128)*C]: chunk-c data contiguous per p
        R, C = a.shape
        return np.ascontiguousarray(
            a.reshape(R // 128, 128, C).transpose(1, 0, 2).reshape(128, -1)
        )

    in_maps = []
    for c in range(N_CORES):
        b, g = c // 2, c % 2
        gsl = slice(512 * g, 512 * (g + 1))
        in_maps.append(
            {
                "xq_t": np.ascontiguousarray(query[b].T.astype(bf)),
                "xk_t": np.ascontiguousarray(key[b].T.astype(bf)),
                "xv_t": np.ascontiguousarray(value[b].T.astype(bf)),
                "wq_t": ileave(Wq[gsl, :].T.astype(bf)),
                "wk_t": ileave(Wk[gsl, :].T.astype(bf)),
                "wv_t": ileave(Wv[gsl, :].T.astype(bf)),
                "wo_s": ileave(Wo[:, gsl].T.astype(bf)),
                "maskd": maskd,
                "sel8": sel8,
                "ones_col": ones_col,
            }
        )
    return in_maps


def _gather(results, bo, B):
    out = np.empty((B, S, D), np.float32)
    for b in range(B):
        out[b] = (
            results[2 * b]["out_p"].astype(np.float32)
            + results[2 * b + 1]["out_p"].astype(np.float32)
            + np.asarray(bo)[None, :]
        )
    return out


def _reference_fallback(query, key, value, mask, Wq, Wk, Wv, Wo, bo):
    B = query.shape[0]
    H = 16
    dk = D // H
    q = np.asarray(query, np.float32)
    k = np.asarray(key, np.float32)
    v = np.asarray(value, np.float32)

    def proj(x, W):
        return (x @ W.T).reshape(B, S, H, dk).transpose(0, 2, 1, 3)

    Q, K, V = proj(q, Wq), proj(k, Wk), proj(v, Wv)
    sc = np.einsum("bhqd,bhkd->bhqk", Q, K) / np.sqrt(np.float32(dk))
    sc = np.where(np.asarray(mask) == 0, np.float32(-1e9), sc)
    sc = sc - sc.max(axis=-1, keepdims=True)
    a = np.exp(sc)
    a = a / a.sum(axis=-1, keepdims=True)
    o = np.einsum("bhqk,bhkd->bhqd", a, V).transpose(0, 2, 1, 3).reshape(B, S, D)
    return (o @ np.asarray(Wo).T + np.asarray(bo)).astype(np.float32)


def kernel(query, key, value, mask, Wq, Wk, Wv, Wo, bo):
    query = np.asarray(query, np.float32)
    key = np.asarray(key, np.float32)
    value = np.asarray(value, np.float32)
    Wq, Wk, Wv, Wo = (np.asarray(w, np.float32) for w in (Wq, Wk, Wv, Wo))
    in_maps = _host_prep(query, key, value, mask, Wq, Wk, Wv, Wo)
    if in_maps is None:  # non-causal mask: host fallback
        return _reference_fallback(query, key, value, mask, Wq, Wk, Wv, Wo, bo)
    nc = _get(1)
    res = run_bass_kernel_spmd(nc, in_maps, list(range(N_CORES)))
    return _gather(res.results, bo, query.shape[0])


def run_spmd(in_maps, repeat=1):
    """For test.py: run prebuilt kernel, return BassKernelResults."""
    nc = _get(repeat)
    return run_bass_kernel_spmd(nc, in_maps, list(range(N_CORES)))


def host_prep(*args, **kw):
    return _host_prep(*args, **kw)


def gather(results, bo, B=4):
    return _gather(results, bo, B)

